# revision 17
# baseline (speedup 1.0000x reference)
"""KGE (TransR-style) loss kernel for Trainium2, 8 NeuronCores.

Conservative fast variant: only instruction forms already proven on this
hardware by the v1 kernel (InstDMACopy indirect gathers, f32 PE
transpose/matmul, ACT activation with scale/accum, DVE
tensor_tensor/tensor_scalar/reduce_sum), restructured:
  - ONE fused multi-index indirect DMA per chunk (the per-gather 1us SWDGE
    descriptor-generation cost of v1 amortizes over 9-12 rows-columns)
  - per-relation GG_k = [0.5*G_k | W_k@r_k] precomputed on host,
    G = W@W^T, using
      neg_score - pos_score = sum((S'@[0.5G|g]) * [T'|-1]) per row,
      S' = Nt - Pt, T' = Pt + Nt - 2H
    (one 129-wide matmul per block instead of four 128-wide ones)
  - raw dm matrix returned; host applies mask/softplus/reg/mean.
"""

import os
from contextlib import ExitStack

import numpy as np

import concourse.bass as bass
import concourse.tile as tile
from concourse import bacc, mybir
from concourse.masks import make_identity

M = 8192
E = 128
C = E + 1
N_ENT = 500000
N_REL = 64
LAM = 1e-5
P = 128
N_CORES = 8
NCH = 4
f32 = mybir.dt.float32
i32 = mybir.dt.int32

_cache = {}


def _build(NB: int):
    assert NB % NCH == 0
    CHUNKS = [4, 3, 3, 2] if NB == 12 else [NB // NCH] * NCH
    OFFS = [sum(CHUNKS[:i]) for i in range(len(CHUNKS) + 1)]

    nc = bacc.Bacc(
        "TRN2",
        target_bir_lowering=False,
        debug=False,
        num_devices=N_CORES,
    )

    ent = nc.dram_tensor("ent", (N_ENT, E), f32, kind="ExternalInput").ap()
    idx = nc.dram_tensor("idx", (P, NB * 3), i32, kind="ExternalInput").ap()
    ggd = nc.dram_tensor("gg", (P, NB * C), f32, kind="ExternalInput").ap()
    out = nc.dram_tensor("out", (P, NB), f32, kind="ExternalOutput").ap()

    with tile.TileContext(nc) as tc, ExitStack() as ctx:
        const = ctx.enter_context(tc.tile_pool(name="const", bufs=1))
        up = ctx.enter_context(tc.tile_pool(name="up", bufs=2))
        hp = ctx.enter_context(tc.tile_pool(name="hp", bufs=2))
        sp = ctx.enter_context(tc.tile_pool(name="sp", bufs=2))
        tp = ctx.enter_context(tc.tile_pool(name="tp", bufs=2))
        sb3 = ctx.enter_context(tc.tile_pool(name="sb3", bufs=4))
        scrp = ctx.enter_context(tc.tile_pool(name="scrp", bufs=4))
        stp = ctx.enter_context(tc.tile_pool(name="stp", bufs=4, space="PSUM"))
        zp = ctx.enter_context(tc.tile_pool(name="zp", bufs=4, space="PSUM"))

        iden = const.tile([P, P], f32)
        gg_sb = const.tile([P, NB * C], f32)
        idx_sb = const.tile([P, NB * 3], i32)
        x_all = const.tile([P, NB * 3 * E], f32)
        dmcols = const.tile([P, NB], f32)

        nc.sync.dma_start(out=idx_sb[:], in_=idx[:])

        # the HW SWDGE takes ONE index per partition and reads
        # out.free_size contiguous elements: one gather per (block, tensor)
        def _gather(c):
            for k in range(OFFS[c] * 3, OFFS[c + 1] * 3):
                nc.gpsimd.indirect_dma_start(
                    out=x_all[:, k * E : (k + 1) * E],
                    out_offset=None,
                    in_=ent[:],
                    in_offset=bass.IndirectOffsetOnAxis(
                        ap=idx_sb[:, k : k + 1], axis=0
                    ),
                )

        _gather(0)
        make_identity(nc, iden[:])
        nc.sync.dma_start(out=gg_sb[:], in_=ggd[:])
        for c in range(1, len(CHUNKS)):
            _gather(c)

        for c in range(len(CHUNKS)):
            BPC = CHUNKS[c]
            xc = x_all[:, OFFS[c] * 3 * E : OFFS[c + 1] * 3 * E].rearrange(
                "p (b t e) -> p t b e", b=BPC, t=3, e=E
            )
            hch = xc[:, 0]
            pch = xc[:, 1]
            nch = xc[:, 2]

            s_c = sp.tile([P, BPC * E], f32, tag="s")
            sv = s_c[:].rearrange("p (b e) -> p b e", b=BPC, e=E)
            nc.vector.tensor_tensor(
                out=sv, in0=nch, in1=pch, op=mybir.AluOpType.subtract
            )

            ueng = nc.vector if c < 2 else nc.gpsimd
            u = up.tile([P, BPC * E], f32, tag="u")
            uv = u[:].rearrange("p (b e) -> p b e", b=BPC, e=E)
            ueng.tensor_tensor(out=uv, in0=pch, in1=nch, op=mybir.AluOpType.add)

            # h2 = 2H on ACT (activation Copy with scale)
            h2 = hp.tile([P, BPC * E], f32, tag="h2")
            h2v = h2[:].rearrange("p (b e) -> p b e", b=BPC, e=E)
            nc.scalar.activation(
                out=h2v, in_=hch, func=mybir.ActivationFunctionType.Copy,
                scale=2.0,
            )

            # taug chunk = [T' | -1] blocks, T' = U - 2H
            taug = tp.tile([P, BPC * C], f32, tag="taug")
            nc.gpsimd.memset(taug[:], -1.0)
            tv = taug[:].rearrange("p (b c1) -> p b c1", b=BPC, c1=C)[:, :, 0:E]
            nc.vector.tensor_tensor(
                out=tv, in0=uv, in1=h2v, op=mybir.AluOpType.subtract
            )

            for b in range(OFFS[c], OFFS[c + 1]):
                with tc.high_priority(offset=4000 - b * 20):
                    lb = b - OFFS[c]
                    st_ps = stp.tile([P, P], f32, tag="stps")
                    nc.tensor.transpose(
                        out=st_ps[:], in_=s_c[:, lb * E : (lb + 1) * E],
                        identity=iden[:],
                    )
                    st_sb = sb3.tile([P, P], f32, tag="st1")
                    nc.scalar.copy(st_sb[:], st_ps[:])
                    z_ps = zp.tile([P, C], f32, tag="z")
                    nc.tensor.matmul(
                        out=z_ps[:], lhsT=st_sb[:],
                        rhs=gg_sb[:, b * C : (b + 1) * C],
                        start=True, stop=True,
                    )
                    v = scrp.tile([P, C], f32, tag="v")
                    nc.vector.tensor_tensor(
                        out=v[:], in0=z_ps[:], in1=taug[:, lb * C : (lb + 1) * C],
                        op=mybir.AluOpType.mult,
                    )
                    if b % 2 == 0:
                        nc.vector.reduce_sum(
                            out=dmcols[:, b : b + 1], in_=v[:],
                            axis=mybir.AxisListType.X,
                        )
                    else:
                        vj = scrp.tile([P, C], f32, tag="vj")
                        nc.scalar.activation(
                            out=vj[:], in_=v[:],
                            func=mybir.ActivationFunctionType.Copy,
                            accum_out=dmcols[:, b : b + 1],
                        )

        nc.sync.dma_start(out=out[:], in_=dmcols[:])

    nc.compile()
    return nc


def _plan(h, r, pos_t, neg_t, relation_weight, relation_embed):
    order = np.argsort(r, kind="stable")
    counts = np.bincount(r, minlength=N_REL)
    blocks = []
    pos = 0
    for k in range(N_REL):
        c = int(counts[k])
        ids = order[pos : pos + c]
        pos += c
        for s in range(0, c, P):
            blocks.append((k, ids[s : s + P]))
    nb = -(-len(blocks) // N_CORES)
    nb = -(-nb // NCH) * NCH
    while len(blocks) < nb * N_CORES:
        blocks.append((0, np.empty(0, np.int64)))

    gg_rel = np.zeros((N_REL, E, C), np.float32)
    gg_rel[:, :, :E] = 0.5 * np.einsum(
        "ker,kfr->kef", relation_weight, relation_weight, optimize=True
    )
    gg_rel[:, :, E] = np.einsum("ker,kr->ke", relation_weight, relation_embed)

    maps = []
    masks = []
    for c in range(N_CORES):
        core_blocks = blocks[c * nb : (c + 1) * nb]
        idx3 = np.zeros((P, nb, 3), np.int32)
        gg = np.zeros((P, nb, C), np.float32)
        mask = np.zeros((P, nb), bool)
        for b, (k, ids) in enumerate(core_blocks):
            n = len(ids)
            if n:
                idx3[:n, b, 0] = h[ids]
                idx3[:n, b, 1] = pos_t[ids]
                idx3[:n, b, 2] = neg_t[ids]
                gg[:, b, :] = gg_rel[k]
            mask[:n, b] = True
        maps.append(
            {
                "idx": np.ascontiguousarray(idx3.reshape(P, nb * 3)),
                "gg": np.ascontiguousarray(gg.reshape(P, nb * C)),
            }
        )
        masks.append(mask)
    return nb, maps, masks, counts


def _finish(outs, masks, h, r, pos_t, neg_t, ent, re):
    total = 0.0
    for c in range(N_CORES):
        dm = np.asarray(outs[c], np.float64)
        y = dm[masks[c]]
        total += (np.maximum(y, 0.0) + np.log1p(np.exp(-np.abs(y)))).sum()
    ent64 = ent.astype(np.float64)
    reg = (
        np.sum(ent64[h] ** 2) + np.sum(ent64[pos_t] ** 2) + np.sum(ent64[neg_t] ** 2)
        + np.sum(re.astype(np.float64)[r] ** 2)
    )
    total += 0.5 * LAM * reg
    return np.float32(total / M)


def kernel(h, r, pos_t, neg_t, entity_embed, relation_embed, relation_weight):
    h = np.asarray(h).astype(np.int32)
    r = np.asarray(r).astype(np.int32)
    pos_t = np.asarray(pos_t).astype(np.int32)
    neg_t = np.asarray(neg_t).astype(np.int32)
    ent = np.ascontiguousarray(np.asarray(entity_embed, dtype=np.float32))
    re = np.ascontiguousarray(np.asarray(relation_embed, dtype=np.float32))
    rw = np.ascontiguousarray(np.asarray(relation_weight, dtype=np.float32))

    nb, maps, masks, counts = _plan(h, r, pos_t, neg_t, rw, re)
    if nb not in _cache:
        _cache[nb] = _build(nb)
    nc = _cache[nb]

    in_maps = [{"ent": ent, **maps[c]} for c in range(N_CORES)]

    if os.environ.get("KGE_SIM"):
        from concourse.bass_interp import CoreSim

        outs = []
        for c in range(N_CORES):
            sim = CoreSim(nc, trace=False)
            for name, arr in in_maps[c].items():
                sim.tensor(name)[:] = arr
            sim.simulate()
            outs.append(np.array(sim.tensor("out")))
        return _finish(outs, masks, h, r, pos_t, neg_t, ent, re)

    from concourse.bass_utils import run_bass_kernel_spmd

    res = run_bass_kernel_spmd(nc, in_maps, core_ids=list(range(N_CORES)))
    outs = [res.results[c]["out"] for c in range(N_CORES)]
    return _finish(outs, masks, h, r, pos_t, neg_t, ent, re)


# revision 19
# speedup vs baseline: 1.0349x; 1.0349x over previous
"""KGE (TransR-style) loss kernel for Trainium2, 8 NeuronCores.

Conservative fast variant: only instruction forms already proven on this
hardware by the v1 kernel (InstDMACopy indirect gathers, f32 PE
transpose/matmul, ACT activation with scale/accum, DVE
tensor_tensor/tensor_scalar/reduce_sum), restructured:
  - ONE fused multi-index indirect DMA per chunk (the per-gather 1us SWDGE
    descriptor-generation cost of v1 amortizes over 9-12 rows-columns)
  - per-relation GG_k = [0.5*G_k | W_k@r_k] precomputed on host,
    G = W@W^T, using
      neg_score - pos_score = sum((S'@[0.5G|g]) * [T'|-1]) per row,
      S' = Nt - Pt, T' = Pt + Nt - 2H
    (one 129-wide matmul per block instead of four 128-wide ones)
  - raw dm matrix returned; host applies mask/softplus/reg/mean.
"""

import os
from contextlib import ExitStack

import numpy as np

import concourse.bass as bass
import concourse.tile as tile
from concourse import bacc, mybir

M = 8192
E = 128
C = E + 1
N_ENT = 500000
N_REL = 64
LAM = 1e-5
P = 128
N_CORES = 8
NCH = 4
f32 = mybir.dt.float32
i32 = mybir.dt.int32

_cache = {}


def _build(NB: int):
    assert NB % NCH == 0
    CHUNKS = [4, 4, 3, 1] if NB == 12 else [NB // NCH] * NCH
    OFFS = [sum(CHUNKS[:i]) for i in range(len(CHUNKS) + 1)]

    nc = bacc.Bacc(
        "TRN2",
        target_bir_lowering=False,
        debug=False,
        num_devices=N_CORES,
    )

    ent = nc.dram_tensor("ent", (N_ENT, E), f32, kind="ExternalInput").ap()
    idx = nc.dram_tensor("idx", (P, NB * 3), i32, kind="ExternalInput").ap()
    ggd = nc.dram_tensor("gg", (P, NB * C), f32, kind="ExternalInput").ap()
    idend = nc.dram_tensor("iden", (P, P), f32, kind="ExternalInput").ap()
    out = nc.dram_tensor("out", (P, NB), f32, kind="ExternalOutput").ap()

    with tile.TileContext(nc) as tc, ExitStack() as ctx:
        const = ctx.enter_context(tc.tile_pool(name="const", bufs=1))
        up = ctx.enter_context(tc.tile_pool(name="up", bufs=2))
        hp = ctx.enter_context(tc.tile_pool(name="hp", bufs=2))
        sp = ctx.enter_context(tc.tile_pool(name="sp", bufs=2))
        tp = ctx.enter_context(tc.tile_pool(name="tp", bufs=2))
        sb3 = ctx.enter_context(tc.tile_pool(name="sb3", bufs=4))
        scrp = ctx.enter_context(tc.tile_pool(name="scrp", bufs=4))
        stp = ctx.enter_context(tc.tile_pool(name="stp", bufs=4, space="PSUM"))
        zp = ctx.enter_context(tc.tile_pool(name="zp", bufs=4, space="PSUM"))

        iden = const.tile([P, P], f32)
        gg_sb = const.tile([P, NB * C], f32)
        idx_sb = const.tile([P, NB * 3], i32)
        x_all = const.tile([P, NB * 3 * E], f32)
        dmcols = const.tile([P, NB], f32)

        nc.sync.dma_start(out=idx_sb[:], in_=idx[:])

        # the HW SWDGE takes ONE index per partition and reads
        # out.free_size contiguous elements: one gather per (block, tensor)
        def _gather(c):
            for k in range(OFFS[c] * 3, OFFS[c + 1] * 3):
                nc.gpsimd.indirect_dma_start(
                    out=x_all[:, k * E : (k + 1) * E],
                    out_offset=None,
                    in_=ent[:],
                    in_offset=bass.IndirectOffsetOnAxis(
                        ap=idx_sb[:, k : k + 1], axis=0
                    ),
                )

        nc.sync.dma_start(out=iden[:], in_=idend[:])
        nc.sync.dma_start(out=gg_sb[:], in_=ggd[:])
        for c in range(len(CHUNKS)):
            _gather(c)

        for c in range(len(CHUNKS)):
            BPC = CHUNKS[c]
            xc = x_all[:, OFFS[c] * 3 * E : OFFS[c + 1] * 3 * E].rearrange(
                "p (b t e) -> p t b e", b=BPC, t=3, e=E
            )
            hch = xc[:, 0]
            pch = xc[:, 1]
            nch = xc[:, 2]

            s_c = sp.tile([P, BPC * E], f32, tag="s")
            sv = s_c[:].rearrange("p (b e) -> p b e", b=BPC, e=E)
            nc.vector.tensor_tensor(
                out=sv, in0=nch, in1=pch, op=mybir.AluOpType.subtract
            )

            u = up.tile([P, BPC * E], f32, tag="u")
            uv = u[:].rearrange("p (b e) -> p b e", b=BPC, e=E)
            nc.vector.tensor_tensor(out=uv, in0=pch, in1=nch, op=mybir.AluOpType.add)

            # h2 = 2H on ACT (activation Copy with scale)
            h2 = hp.tile([P, BPC * E], f32, tag="h2")
            h2v = h2[:].rearrange("p (b e) -> p b e", b=BPC, e=E)
            nc.scalar.activation(
                out=h2v, in_=hch, func=mybir.ActivationFunctionType.Copy,
                scale=2.0,
            )

            # taug chunk = [T' | -1] blocks, T' = U - 2H
            taug = tp.tile([P, BPC * C], f32, tag="taug")
            nc.vector.memset(taug[:], -1.0)
            tv = taug[:].rearrange("p (b c1) -> p b c1", b=BPC, c1=C)[:, :, 0:E]
            nc.vector.tensor_tensor(
                out=tv, in0=uv, in1=h2v, op=mybir.AluOpType.subtract
            )

            for b in range(OFFS[c], OFFS[c + 1]):
                with tc.high_priority(offset=4000 - b * 20):
                    lb = b - OFFS[c]
                    st_ps = stp.tile([P, P], f32, tag="stps")
                    nc.tensor.transpose(
                        out=st_ps[:], in_=s_c[:, lb * E : (lb + 1) * E],
                        identity=iden[:],
                    )
                    st_sb = sb3.tile([P, P], f32, tag="st1")
                    nc.scalar.copy(st_sb[:], st_ps[:])
                    z_ps = zp.tile([P, C], f32, tag="z")
                    nc.tensor.matmul(
                        out=z_ps[:], lhsT=st_sb[:],
                        rhs=gg_sb[:, b * C : (b + 1) * C],
                        start=True, stop=True,
                    )
                    v = scrp.tile([P, C], f32, tag="v")
                    nc.vector.tensor_tensor(
                        out=v[:], in0=z_ps[:], in1=taug[:, lb * C : (lb + 1) * C],
                        op=mybir.AluOpType.mult,
                    )
                    if b % 2 == 0:
                        nc.vector.reduce_sum(
                            out=dmcols[:, b : b + 1], in_=v[:],
                            axis=mybir.AxisListType.X,
                        )
                    else:
                        vj = scrp.tile([P, C], f32, tag="vj")
                        nc.scalar.activation(
                            out=vj[:], in_=v[:],
                            func=mybir.ActivationFunctionType.Copy,
                            accum_out=dmcols[:, b : b + 1],
                        )

        nc.sync.dma_start(out=out[:], in_=dmcols[:])

    nc.compile()
    return nc


def _plan(h, r, pos_t, neg_t, relation_weight, relation_embed):
    order = np.argsort(r, kind="stable")
    counts = np.bincount(r, minlength=N_REL)
    blocks = []
    pos = 0
    for k in range(N_REL):
        c = int(counts[k])
        ids = order[pos : pos + c]
        pos += c
        for s in range(0, c, P):
            blocks.append((k, ids[s : s + P]))
    nb = -(-len(blocks) // N_CORES)
    nb = -(-nb // NCH) * NCH
    while len(blocks) < nb * N_CORES:
        blocks.append((0, np.empty(0, np.int64)))

    gg_rel = np.zeros((N_REL, E, C), np.float32)
    gg_rel[:, :, :E] = 0.5 * np.einsum(
        "ker,kfr->kef", relation_weight, relation_weight, optimize=True
    )
    gg_rel[:, :, E] = np.einsum("ker,kr->ke", relation_weight, relation_embed)

    maps = []
    masks = []
    for c in range(N_CORES):
        core_blocks = blocks[c * nb : (c + 1) * nb]
        idx3 = np.zeros((P, nb, 3), np.int32)
        gg = np.zeros((P, nb, C), np.float32)
        mask = np.zeros((P, nb), bool)
        for b, (k, ids) in enumerate(core_blocks):
            n = len(ids)
            if n:
                idx3[:n, b, 0] = h[ids]
                idx3[:n, b, 1] = pos_t[ids]
                idx3[:n, b, 2] = neg_t[ids]
                gg[:, b, :] = gg_rel[k]
            mask[:n, b] = True
        maps.append(
            {
                "idx": np.ascontiguousarray(idx3.reshape(P, nb * 3)),
                "gg": np.ascontiguousarray(gg.reshape(P, nb * C)),
            }
        )
        masks.append(mask)
    return nb, maps, masks, counts


def _finish(outs, masks, h, r, pos_t, neg_t, ent, re):
    total = 0.0
    for c in range(N_CORES):
        dm = np.asarray(outs[c], np.float64)
        y = dm[masks[c]]
        total += (np.maximum(y, 0.0) + np.log1p(np.exp(-np.abs(y)))).sum()
    ent64 = ent.astype(np.float64)
    reg = (
        np.sum(ent64[h] ** 2) + np.sum(ent64[pos_t] ** 2) + np.sum(ent64[neg_t] ** 2)
        + np.sum(re.astype(np.float64)[r] ** 2)
    )
    total += 0.5 * LAM * reg
    return np.float32(total / M)


def kernel(h, r, pos_t, neg_t, entity_embed, relation_embed, relation_weight):
    h = np.asarray(h).astype(np.int32)
    r = np.asarray(r).astype(np.int32)
    pos_t = np.asarray(pos_t).astype(np.int32)
    neg_t = np.asarray(neg_t).astype(np.int32)
    ent = np.ascontiguousarray(np.asarray(entity_embed, dtype=np.float32))
    re = np.ascontiguousarray(np.asarray(relation_embed, dtype=np.float32))
    rw = np.ascontiguousarray(np.asarray(relation_weight, dtype=np.float32))

    nb, maps, masks, counts = _plan(h, r, pos_t, neg_t, rw, re)
    if nb not in _cache:
        _cache[nb] = _build(nb)
    nc = _cache[nb]

    iden_host = np.eye(P, dtype=np.float32)
    in_maps = [{"ent": ent, "iden": iden_host, **maps[c]} for c in range(N_CORES)]

    if os.environ.get("KGE_SIM"):
        from concourse.bass_interp import CoreSim

        outs = []
        for c in range(N_CORES):
            sim = CoreSim(nc, trace=False)
            for name, arr in in_maps[c].items():
                sim.tensor(name)[:] = arr
            sim.simulate()
            outs.append(np.array(sim.tensor("out")))
        return _finish(outs, masks, h, r, pos_t, neg_t, ent, re)

    from concourse.bass_utils import run_bass_kernel_spmd

    res = run_bass_kernel_spmd(nc, in_maps, core_ids=list(range(N_CORES)))
    outs = [res.results[c]["out"] for c in range(N_CORES)]
    return _finish(outs, masks, h, r, pos_t, neg_t, ent, re)
